# revision 40
# baseline (speedup 1.0000x reference)
"""Causal self-attention (GQA + RoPE + QK-RMSNorm) on 8 trn2 NeuronCores.

Reference (B=2, T=2048, C=2048, 16 q-heads / 4 kv-heads, head_dim 128):
    q = rms_norm(rope(x @ Wq)) / sqrt(128); k = rms_norm(rope(x @ Wk))
    att = softmax_causal(q k^T / sqrt(128)); y = (att @ v) @ Wp

Sharding: core = 4*b + g  (b = batch 0..1, g = head-group 0..3).
Each core computes q-heads 4g..4g+3 (all mapping to kv-head g under the GQA
repeat), attends over the full causal sequence of its batch, AllGathers its
attention output per 512-query chunk across the 4 cores of its batch, and
computes a distinct 512-column slice of the output projection per chunk.

On-chip layout is "transposed activations": X^T, Q^T, K^T, S^T, P^T, Y^T all
[feature, t] so every matmul has contraction on the partition axis.
Softmax runs without max subtraction (|logit| <= 1 by construction).

Engine split: PE streams bf16 matmuls back-to-back; Scalar does PSUM->bf16
casts and the paired EXPs (two heads per instruction, shared per-key k-norm
scale); DVE does RoPE/masks/denominator accumulation in bf16 at 4x rate;
GpSimd does partition reductions (q-norm sumsq, softmax denominators) and
triggers collectives. Softmax denominators need no PE row-sum matmuls.
"""

import ml_dtypes
import numpy as np

B, T, C = 2, 2048, 2048
NH, NKV, HD = 16, 4, 128
G = 4  # q-heads per core
EPS = 1e-6
NCB = C // 128  # 16 contraction blocks
NTCH = T // 512  # 4 t-chunks
NTKB = T // 128  # 16 key blocks

_CACHE = {}


def _build():
    import concourse.mybir as mybir
    import concourse.tile as tile
    from concourse import bacc
    from contextlib import ExitStack

    F32 = mybir.dt.float32
    BF16 = mybir.dt.bfloat16
    AF = mybir.ActivationFunctionType

    nc = bacc.Bacc(None, target_bir_lowering=False, num_devices=8)

    # all inputs arrive partition-major (host pre-shuffles) so DMA
    # descriptors are 4-16KB contiguous runs per partition
    xT = nc.dram_tensor("xT", [128, NTCH, NCB, 512], BF16, kind="ExternalInput")
    wq = nc.dram_tensor("wq", [128, NCB * G * HD], BF16, kind="ExternalInput")
    wk = nc.dram_tensor("wk", [128, NCB * HD], BF16, kind="ExternalInput")
    wv = nc.dram_tensor("wv", [128, NCB * HD], BF16, kind="ExternalInput")
    wp = nc.dram_tensor("wp", [128, NCB * G * HD], BF16, kind="ExternalInput")
    cosT = nc.dram_tensor("cosT", [64, T], BF16, kind="ExternalInput")
    sinT = nc.dram_tensor("sinT", [64, T], BF16, kind="ExternalInput")
    masks = nc.dram_tensor("masks", [4, 128, 512], BF16, kind="ExternalInput")
    outT = nc.dram_tensor("outT", [G * HD, T], F32, kind="ExternalOutput")

    with tile.TileContext(nc) as tc:
        with ExitStack() as outer:
            dram = outer.enter_context(tc.tile_pool(name="dram", bufs=1, space="DRAM"))
            # one AG per chunk; the last chunk is split by head-pair so its
            # first half overlaps the final attention pair
            ag_ins = {}
            ag_outs = {}
            for c in range(3):
                ag_ins[(c, 0)] = dram.tile([512, 512], BF16, name=f"ag_in_{c}")
                ag_outs[(c, 0)] = dram.tile([2048, 512], BF16, name=f"ag_out_{c}")
            for p in range(2):
                ag_ins[(3, p)] = dram.tile([256, 512], BF16, name=f"ag_in_3_{p}")
                ag_outs[(3, p)] = dram.tile([1024, 512], BF16, name=f"ag_out_3_{p}")

            consts = outer.enter_context(tc.tile_pool(name="consts", bufs=1))
            rk_col = consts.tile([128, NTKB], F32)
            eps_k = consts.tile([128, 1], F32)
            nc.vector.memset(eps_k[:], EPS)
            eps_q = consts.tile([128, 1], F32)
            nc.vector.memset(eps_q[:], float(HD * HD) * EPS)
            ones_bf = consts.tile([128, 1], BF16)
            nc.vector.memset(ones_bf[:], 1.0)
            allones = consts.tile([128, 128], BF16)
            nc.vector.memset(allones[:], 1.0)
            ident_bf = consts.tile([128, 128], BF16)
            from concourse.masks import make_identity
            make_identity(nc, ident_bf[:])

            trig = outer.enter_context(tc.tile_pool(name="trig", bufs=1))
            cos_sb = trig.tile([128, T], BF16)
            sin_sb = trig.tile([128, T], BF16)
            masks_sb = trig.tile([128, 4, 512], BF16)

            wp_pool = outer.enter_context(tc.tile_pool(name="wpp", bufs=1))
            wp_sb = wp_pool.tile([128, NCB, G * HD], BF16)

            acts = outer.enter_context(tc.tile_pool(name="acts", bufs=1))
            qT_sb = acts.tile([128, G, T], BF16)
            kT_sb = acts.tile([128, T], BF16)
            v_sb = acts.tile([128, NTKB, HD], BF16)

            # ---- phase 1: Q/K/V projections + RoPE + RMS-norm ----
            with ExitStack() as sA:
                wpool = sA.enter_context(tc.tile_pool(name="w", bufs=1))
                wq_sb = wpool.tile([128, NCB, G * HD], BF16)
                wk_sb = wpool.tile([128, NCB, HD], BF16)
                wv_sb = wpool.tile([128, NCB, HD], BF16)
                # startup-critical loads first, spread across two sequencers
                nc.sync.dma_start(out=wk_sb[:], in_=wk[:])

                xt_pool = sA.enter_context(tc.tile_pool(name="xt", bufs=2))

                xt0a = xt_pool.tile([128, 8, 512], BF16, tag="xta")
                xt0b = xt_pool.tile([128, 8, 512], BF16, tag="xtb")
                nc.sync.dma_start(out=xt0a[:], in_=xT[:, 0, 0:8, :])
                nc.sync.dma_start(out=xt0b[:], in_=xT[:, 0, 8:16, :])

                nc.scalar.dma_start(out=cos_sb[0:64, :], in_=cosT[:])
                nc.scalar.dma_start(out=cos_sb[64:128, :], in_=cosT[:])
                nc.scalar.dma_start(out=sin_sb[64:128, :], in_=sinT[:])
                nc.vector.tensor_scalar_mul(sin_sb[0:64, :], sin_sb[64:128, :], -1.0)
                nc.scalar.dma_start(out=wv_sb[:], in_=wv[:])
                nc.scalar.dma_start(out=wq_sb[:], in_=wq[:])

                tmp = sA.enter_context(tc.tile_pool(name="tmp", bufs=3))
                nrm = sA.enter_context(tc.tile_pool(name="nrm", bufs=2))
                psk = sA.enter_context(tc.tile_pool(name="psk", bufs=1, space="PSUM"))
                psv = sA.enter_context(tc.tile_pool(name="psv", bufs=1, space="PSUM"))
                psq = sA.enter_context(tc.tile_pool(name="psq", bufs=2, space="PSUM"))
                pstr = sA.enter_context(tc.tile_pool(name="pstr", bufs=1, space="PSUM"))
                pskc = sA.enter_context(tc.tile_pool(name="pskc", bufs=1, space="PSUM"))
                psn = sA.enter_context(tc.tile_pool(name="psn", bufs=2, space="PSUM"))

                def rope_bf(dst, src_ps, tcs, tag):
                    """dst(bf16) = rope(src_ps); cast on Scalar, half-swap
                    copies + mul/mul/add on DVE in bf16."""
                    cast = tmp.tile([128, 512], BF16, tag=f"{tag}c")
                    nc.scalar.activation(out=cast[:], in_=src_ps[:], func=AF.Copy)
                    sw = tmp.tile([128, 512], BF16, tag=f"{tag}s")
                    nc.vector.tensor_copy(out=sw[0:64, :], in_=cast[64:128, :])
                    nc.vector.tensor_copy(out=sw[64:128, :], in_=cast[0:64, :])
                    t1 = tmp.tile([128, 512], BF16, tag=f"{tag}1")
                    nc.vector.tensor_mul(t1[:], cast[:], cos_sb[:, tcs])
                    nc.vector.tensor_mul(sw[:], sw[:], sin_sb[:, tcs])
                    nc.vector.tensor_add(dst, t1[:], sw[:])

                for tch in range(NTCH):
                    tcs = slice(512 * tch, 512 * tch + 512)
                    if tch == 1:
                        # non-startup-critical loads, issued once tch0 is rolling
                        nc.scalar.dma_start(
                            out=masks_sb[:], in_=masks.rearrange("d p m -> p d m")
                        )
                        nc.scalar.dma_start(out=wp_sb[:], in_=wp[:])
                    if tch == 0:
                        xta, xtb = xt0a, xt0b
                    else:
                        xta = xt_pool.tile([128, 8, 512], BF16, tag="xta")
                        xtb = xt_pool.tile([128, 8, 512], BF16, tag="xtb")
                        nc.sync.dma_start(out=xta[:], in_=xT[:, tch, 0:8, :])
                        nc.sync.dma_start(out=xtb[:], in_=xT[:, tch, 8:16, :])

                    def xt(cb):
                        return xta[:, cb, :] if cb < 8 else xtb[:, cb - 8, :]

                    # K^T chunk [128 d, 512 t]
                    ps_k = psk.tile([128, 512], F32, tag="psk")
                    for cb in range(NCB):
                        nc.tensor.matmul(
                            ps_k[:], wk_sb[:, cb, :], xt(cb),
                            start=(cb == 0), stop=(cb == NCB - 1),
                        )
                    rope_bf(kT_sb[:, tcs], ps_k, tcs, "k")
                    ksq = tmp.tile([128, 512], BF16, tag="ksq")
                    nc.vector.tensor_mul(ksq[:], kT_sb[:, tcs], kT_sb[:, tcs])

                    # V^T, then PE-transpose each 128-block into v_sb
                    ps_v = psv.tile([128, 512], F32, tag="psv")
                    for cb in range(NCB):
                        nc.tensor.matmul(
                            ps_v[:], wv_sb[:, cb, :], xt(cb),
                            start=(cb == 0), stop=(cb == NCB - 1),
                        )
                    vt_bf = tmp.tile([128, 512], BF16, tag="vtb")
                    nc.scalar.activation(out=vt_bf[:], in_=ps_v[:], func=AF.Copy)

                    # per-key rms via ksq-stationary matmuls (1 moving col)
                    for j in range(4):
                        ps_kc = pskc.tile([128, 1], F32, tag="pskc")
                        nc.tensor.matmul(
                            ps_kc[:], ksq[:, 128 * j : 128 * j + 128], ones_bf[:],
                            start=True, stop=True,
                        )
                        scol = nrm.tile([128, 1], F32, tag="scol")
                        nc.scalar.activation(
                            out=scol[:], in_=ps_kc[:], func=AF.Sqrt,
                            scale=1.0 / HD, bias=eps_k[:],
                        )
                        nc.vector.reciprocal_approx_fast(
                            out=rk_col[:, 4 * tch + j : 4 * tch + j + 1], in_=scol[:]
                        )

                    # Q^T per head [128 d, 512 t]
                    for hq in range(G):
                        ps_q = psq.tile([128, 512], F32, tag="psq")
                        for cb in range(NCB):
                            nc.tensor.matmul(
                                ps_q[:],
                                wq_sb[:, cb, 128 * hq : 128 * hq + 128],
                                xt(cb),
                                start=(cb == 0), stop=(cb == NCB - 1),
                            )
                        if hq == 0:
                            # V transposes slot in here (vt_bf is ready)
                            for tt in range(4):
                                ps_tr = pstr.tile([128, 128], BF16, tag="pstr")
                                nc.tensor.transpose(
                                    ps_tr[:], vt_bf[:, 128 * tt : 128 * tt + 128],
                                    ident_bf[:],
                                )
                                nc.vector.tensor_copy(
                                    out=v_sb[:, 4 * tch + tt, :], in_=ps_tr[:]
                                )
                        qrope = tmp.tile([128, 512], BF16, tag="qr")
                        rope_bf(qrope[:], ps_q, tcs, "q")
                        sq = tmp.tile([128, 512], BF16, tag="sq")
                        nc.vector.tensor_mul(sq[:], qrope[:], qrope[:])
                        # sum over d, broadcast to all partitions, in one matmul
                        ps_ssq = psn.tile([128, 512], F32, tag="psn")
                        nc.tensor.matmul(
                            ps_ssq[:], allones[:], sq[:], start=True, stop=True
                        )
                        # 1/(HD*rms) = 1/sqrt(HD*ss + HD^2*eps)
                        srow = nrm.tile([128, 512], F32, tag="srow")
                        nc.scalar.activation(
                            out=srow[:], in_=ps_ssq[:], func=AF.Sqrt,
                            scale=float(HD), bias=eps_q[:],
                        )
                        rcp = nrm.tile([128, 512], F32, tag="rcpq")
                        nc.vector.reciprocal_approx_fast(out=rcp[:], in_=srow[:])
                        nc.vector.tensor_mul(qT_sb[:, hq, tcs], qrope[:], rcp[:])

            # ---- phase 2: attention (head pairs), AG per chunk; phase 4 ----
            with ExitStack() as sB:
                pt_pool = sB.enter_context(tc.tile_pool(name="pt", bufs=4))
                acc_pool = sB.enter_context(tc.tile_pool(name="acc", bufs=2))
                den_pool = sB.enter_context(tc.tile_pool(name="den", bufs=2))
                yt_pool = sB.enter_context(tc.tile_pool(name="yt", bufs=1))
                ytm_pool = sB.enter_context(tc.tile_pool(name="ytm", bufs=4))
                pss = sB.enter_context(tc.tile_pool(name="pss", bufs=2, space="PSUM"))
                psy = sB.enter_context(tc.tile_pool(name="psy", bufs=4, space="PSUM"))

                yt_tiles = {}
                for c in range(3):
                    yt_tiles[(c, 0)] = yt_pool.tile(
                        [128, NCB, 512], BF16, name=f"ytall_{c}"
                    )
                for p in range(2):
                    yt_tiles[(3, p)] = yt_pool.tile(
                        [128, 8, 512], BF16, name=f"ytall_3_{p}"
                    )
                # triangle mask doubled across the head-pair dim
                mask2 = yt_pool.tile([128, 2, 128], BF16, name="mask2")
                nc.vector.tensor_copy(out=mask2[:, 0, :], in_=masks_sb[:, 0, 0:128])
                nc.vector.tensor_copy(out=mask2[:, 1, :], in_=masks_sb[:, 0, 0:128])

                chunk_order = [0, 1, 2, 3]
                for c in chunk_order:
                    nblk = 4 * c + 4
                    tqs = slice(512 * c, 512 * c + 512)
                    for pair in range(2):
                        h0, h1 = 2 * pair, 2 * pair + 1
                        ps_y0 = psy.tile([128, 512], F32, tag="psy")
                        ps_y1 = psy.tile([128, 512], F32, tag="psy")
                        accum = acc_pool.tile([128, 2, 512], BF16, tag="acc")
                        for tkb in range(nblk):
                            d = tkb - 4 * c
                            ks = slice(128 * tkb, 128 * tkb + 128)
                            # diagonal blocks d>=1: columns < 128d are fully
                            # masked; compute only the live range
                            lo = 128 * d if d >= 1 else 0
                            lv = slice(lo, 512)
                            ps_sp = pss.tile([128, 2, 512], F32, tag="pss")
                            nc.tensor.matmul(
                                ps_sp[:, 0, lv], kT_sb[:, ks],
                                qT_sb[:, h0, 512 * c + lo : 512 * c + 512],
                                start=True, stop=True,
                            )
                            nc.tensor.matmul(
                                ps_sp[:, 1, lv], kT_sb[:, ks],
                                qT_sb[:, h1, 512 * c + lo : 512 * c + 512],
                                start=True, stop=True,
                            )
                            # first exp writes the denominator accumulator tile
                            # directly; it doubles as this block's P
                            pt = (
                                accum
                                if tkb == 0
                                else pt_pool.tile([128, 2, 512], BF16, tag="pt")
                            )
                            nc.scalar.activation(
                                out=pt[:, :, lv], in_=ps_sp[:, :, lv], func=AF.Exp,
                                scale=rk_col[:, tkb : tkb + 1],
                            )
                            if d >= 0:
                                # only the leading 128 live columns need the triangle
                                ms = slice(lo, lo + 128)
                                nc.vector.tensor_mul(
                                    pt[:, :, ms], pt[:, :, ms], mask2[:]
                                )
                            if tkb > 0:
                                nc.vector.tensor_add(
                                    accum[:, :, lv], accum[:, :, lv], pt[:, :, lv]
                                )
                            nc.tensor.matmul(
                                ps_y0[:, lv], v_sb[:, tkb, :], pt[:, 0, lv],
                                start=(tkb == 0), stop=(tkb == nblk - 1),
                                skip_group_check=bool(lo),
                            )
                            nc.tensor.matmul(
                                ps_y1[:, lv], v_sb[:, tkb, :], pt[:, 1, lv],
                                start=(tkb == 0), stop=(tkb == nblk - 1),
                                skip_group_check=bool(lo),
                            )
                        # key-sum broadcast to all partitions via all-ones matmul
                        ps_d0 = psy.tile([128, 512], F32, tag="psy")
                        ps_d1 = psy.tile([128, 512], F32, tag="psy")
                        nc.tensor.matmul(
                            ps_d0[:], allones[:], accum[:, 0, :],
                            start=True, stop=True,
                        )
                        nc.tensor.matmul(
                            ps_d1[:], allones[:], accum[:, 1, :],
                            start=True, stop=True,
                        )
                        rcpd = den_pool.tile([128, 2, 512], F32, tag="rcpd")
                        nc.vector.reciprocal_approx_fast(out=rcpd[:, 0, :], in_=ps_d0[:])
                        nc.vector.reciprocal_approx_fast(out=rcpd[:, 1, :], in_=ps_d1[:])
                        yT = ytm_pool.tile([128, 2, 512], BF16, tag="yT")
                        nc.vector.tensor_mul(yT[:, 0, :], ps_y0[:], rcpd[:, 0, :])
                        nc.vector.tensor_mul(yT[:, 1, :], ps_y1[:], rcpd[:, 1, :])
                        if c == 3:
                            nc.sync.dma_start(
                                out=ag_ins[(3, pair)].rearrange(
                                    "(h p) t -> p h t", p=128
                                ),
                                in_=yT[:],
                            )
                            nc.gpsimd.collective_compute(
                                "AllGather",
                                mybir.AluOpType.bypass,
                                replica_groups=[[0, 1, 2, 3], [4, 5, 6, 7]],
                                ins=[ag_ins[(3, pair)][:]],
                                outs=[ag_outs[(3, pair)][:]],
                            )
                            nc.gpsimd.dma_start(
                                out=yt_tiles[(3, pair)][:],
                                in_=ag_outs[(3, pair)].rearrange(
                                    "(i p) t -> p i t", p=128
                                ),
                            )
                        else:
                            nc.sync.dma_start(
                                out=ag_ins[(c, 0)].rearrange("(h p) t -> p h t", p=128)[
                                    :, 2 * pair : 2 * pair + 2, :
                                ],
                                in_=yT[:],
                            )
                    if c != 3:
                        nc.gpsimd.collective_compute(
                            "AllGather",
                            mybir.AluOpType.bypass,
                            replica_groups=[[0, 1, 2, 3], [4, 5, 6, 7]],
                            ins=[ag_ins[(c, 0)][:]],
                            outs=[ag_outs[(c, 0)][:]],
                        )
                        # gathered-Y load (gpsimd swdge): parks until the AG
                        # lands without blocking sync's yT writes
                        nc.gpsimd.dma_start(
                            out=yt_tiles[(c, 0)][:],
                            in_=ag_outs[(c, 0)].rearrange("(i p) t -> p i t", p=128),
                        )

                # phase 4: out^T chunk = sum_cin Wp^T y^T  (16-block PSUM chains)
                osb_pool = sB.enter_context(tc.tile_pool(name="osb", bufs=3))
                for c in chunk_order:
                    tqs = slice(512 * c, 512 * c + 512)
                    for cob in range(4):
                        ps_o = pss.tile([128, 2, 512], F32, tag="pss")
                        if c != 3:
                            srcs = [(yt_tiles[(c, 0)], i, i) for i in range(NCB)]
                        else:
                            # yt block j of pair-AG = rank j//2, head 2*pair+j%2
                            srcs = [
                                (yt_tiles[(3, p)], j, 4 * (j // 2) + 2 * p + (j % 2))
                                for p in range(2)
                                for j in range(8)
                            ]
                        for n, (yt, j, i) in enumerate(srcs):
                            nc.tensor.matmul(
                                ps_o[:, 0, :],
                                wp_sb[:, i, 128 * cob : 128 * cob + 128],
                                yt[:, j, :],
                                start=(n == 0), stop=(n == NCB - 1),
                            )
                        o_sb = osb_pool.tile([128, 512], F32, tag="osb")
                        nc.scalar.activation(out=o_sb[:], in_=ps_o[:, 0, :], func=AF.Copy)
                        nc.scalar.dma_start(
                            out=outT[128 * cob : 128 * cob + 128, tqs],
                            in_=o_sb[:],
                        )

    nc.compile()
    return nc


def _get_nc():
    if "nc" not in _CACHE:
        _CACHE["nc"] = _build()
    return _CACHE["nc"]


def _shuf_w(w):
    """[C, O] -> [128, 16*O] partition-major (contiguous per-partition runs)."""
    o = w.shape[1]
    return np.ascontiguousarray(
        w.reshape(16, 128, o).transpose(1, 0, 2).reshape(128, 16 * o)
    )


def _shuf_x(xb):
    """x[b] [T, C] -> xT [128, 4, 16, 512]: [p, tch, cb, t']."""
    xt = xb.T.reshape(16, 128, 4, 512)  # [cb, p, tch, t']
    return np.ascontiguousarray(xt.transpose(1, 2, 0, 3))


def kernel(x, cos, sin, Wq, Wk, Wv, Wp):
    from concourse.bass_utils import run_bass_kernel_spmd

    x = np.asarray(x)
    bf16 = ml_dtypes.bfloat16
    cosT = np.ascontiguousarray(np.asarray(cos).T).astype(bf16)
    sinT = np.ascontiguousarray(np.asarray(sin).T).astype(bf16)
    p = np.arange(128, dtype=np.int64)[:, None]
    j = np.arange(512, dtype=np.int64)[None, :]
    masks = np.stack(
        [(j >= p + 128 * d) for d in range(4)], axis=0
    ).astype(bf16)  # [4, 128, 512]

    in_maps = []
    for core in range(8):
        b, g = core // 4, core % 4
        in_maps.append(
            {
                "xT": _shuf_x(x[b]).astype(bf16),
                "wq": _shuf_w(Wq[:, 512 * g : 512 * g + 512]).astype(bf16),
                "wk": _shuf_w(Wk[:, 128 * g : 128 * g + 128]).astype(bf16),
                "wv": _shuf_w(Wv[:, 128 * g : 128 * g + 128]).astype(bf16),
                "wp": _shuf_w(Wp[:, 512 * g : 512 * g + 512]).astype(bf16),
                "cosT": cosT,
                "sinT": sinT,
                "masks": masks,
            }
        )

    nc = _get_nc()
    res = run_bass_kernel_spmd(nc, in_maps, core_ids=list(range(8)), trace=False)

    out = np.empty((B, T, C), dtype=np.float32)
    for core in range(8):
        b, g = core // 4, core % 4
        out[b, :, 512 * g : 512 * g + 512] = res.results[core]["outT"].T
    return out


# revision 41
# speedup vs baseline: 1.0085x; 1.0085x over previous
"""Causal self-attention (GQA + RoPE + QK-RMSNorm) on 8 trn2 NeuronCores.

Reference (B=2, T=2048, C=2048, 16 q-heads / 4 kv-heads, head_dim 128):
    q = rms_norm(rope(x @ Wq)) / sqrt(128); k = rms_norm(rope(x @ Wk))
    att = softmax_causal(q k^T / sqrt(128)); y = (att @ v) @ Wp

Sharding: core = 4*b + g  (b = batch 0..1, g = head-group 0..3).
Each core computes q-heads 4g..4g+3 (all mapping to kv-head g under the GQA
repeat), attends over the full causal sequence of its batch, AllGathers its
attention output per 512-query chunk across the 4 cores of its batch, and
computes a distinct 512-column slice of the output projection per chunk.

On-chip layout is "transposed activations": X^T, Q^T, K^T, S^T, P^T, Y^T all
[feature, t] so every matmul has contraction on the partition axis.
Softmax runs without max subtraction (|logit| <= 1 by construction).

Engine split: PE streams bf16 matmuls back-to-back; Scalar does PSUM->bf16
casts and the paired EXPs (two heads per instruction, shared per-key k-norm
scale); DVE does RoPE/masks/denominator accumulation in bf16 at 4x rate;
GpSimd does partition reductions (q-norm sumsq, softmax denominators) and
triggers collectives. Softmax denominators need no PE row-sum matmuls.
"""

import ml_dtypes
import numpy as np

B, T, C = 2, 2048, 2048
NH, NKV, HD = 16, 4, 128
G = 4  # q-heads per core
EPS = 1e-6
NCB = C // 128  # 16 contraction blocks
NTCH = T // 512  # 4 t-chunks
NTKB = T // 128  # 16 key blocks

_CACHE = {}


def _build():
    import concourse.mybir as mybir
    import concourse.tile as tile
    from concourse import bacc
    from contextlib import ExitStack

    F32 = mybir.dt.float32
    BF16 = mybir.dt.bfloat16
    AF = mybir.ActivationFunctionType

    nc = bacc.Bacc(None, target_bir_lowering=False, num_devices=8)

    # all inputs arrive partition-major (host pre-shuffles) so DMA
    # descriptors are 4-16KB contiguous runs per partition
    xT = nc.dram_tensor("xT", [128, NTCH, NCB, 512], BF16, kind="ExternalInput")
    wq = nc.dram_tensor("wq", [128, NCB * G * HD], BF16, kind="ExternalInput")
    wk = nc.dram_tensor("wk", [128, NCB * HD], BF16, kind="ExternalInput")
    wv = nc.dram_tensor("wv", [128, NCB * HD], BF16, kind="ExternalInput")
    wp = nc.dram_tensor("wp", [128, NCB * G * HD], BF16, kind="ExternalInput")
    cosT = nc.dram_tensor("cosT", [64, T], BF16, kind="ExternalInput")
    sinT = nc.dram_tensor("sinT", [64, T], BF16, kind="ExternalInput")
    masks = nc.dram_tensor("masks", [4, 128, 512], BF16, kind="ExternalInput")
    outT = nc.dram_tensor("outT", [G * HD, T], F32, kind="ExternalOutput")

    with tile.TileContext(nc) as tc:
        with ExitStack() as outer:
            dram = outer.enter_context(tc.tile_pool(name="dram", bufs=1, space="DRAM"))
            # one AG per chunk; the last chunk is split by head-pair so its
            # first half overlaps the final attention pair
            ag_ins = {}
            ag_outs = {}
            for c in range(3):
                ag_ins[(c, 0)] = dram.tile([512, 512], BF16, name=f"ag_in_{c}")
                ag_outs[(c, 0)] = dram.tile([2048, 512], BF16, name=f"ag_out_{c}")
            for p in range(2):
                ag_ins[(3, p)] = dram.tile([256, 512], BF16, name=f"ag_in_3_{p}")
                ag_outs[(3, p)] = dram.tile([1024, 512], BF16, name=f"ag_out_3_{p}")

            consts = outer.enter_context(tc.tile_pool(name="consts", bufs=1))
            rk_col = consts.tile([128, NTKB], F32)
            eps_k = consts.tile([128, 1], F32)
            nc.vector.memset(eps_k[:], EPS)
            eps_q = consts.tile([128, 1], F32)
            nc.vector.memset(eps_q[:], float(HD * HD) * EPS)
            ones_bf = consts.tile([128, 1], BF16)
            nc.vector.memset(ones_bf[:], 1.0)
            allones = consts.tile([128, 128], BF16)
            nc.vector.memset(allones[:], 1.0)
            ident_bf = consts.tile([128, 128], BF16)
            from concourse.masks import make_identity
            make_identity(nc, ident_bf[:])

            trig = outer.enter_context(tc.tile_pool(name="trig", bufs=1))
            cos_sb = trig.tile([128, T], BF16)
            sin_sb = trig.tile([128, T], BF16)
            masks_sb = trig.tile([128, 4, 512], BF16)

            wp_pool = outer.enter_context(tc.tile_pool(name="wpp", bufs=1))
            wp_sb = wp_pool.tile([128, NCB, G * HD], BF16)

            acts = outer.enter_context(tc.tile_pool(name="acts", bufs=1))
            qT_sb = acts.tile([128, G, T], BF16)
            kT_sb = acts.tile([128, T], BF16)
            v_sb = acts.tile([128, NTKB, HD], BF16)

            # ---- phase 1: Q/K/V projections + RoPE + RMS-norm ----
            with ExitStack() as sA:
                wpool = sA.enter_context(tc.tile_pool(name="w", bufs=1))
                wq_sb = wpool.tile([128, NCB, G * HD], BF16)
                wk_sb = wpool.tile([128, NCB, HD], BF16)
                wv_sb = wpool.tile([128, NCB, HD], BF16)
                # startup-critical loads first, spread across two sequencers
                nc.sync.dma_start(out=wk_sb[:], in_=wk[:])

                xt_pool = sA.enter_context(tc.tile_pool(name="xt", bufs=2))

                xt0a = xt_pool.tile([128, 8, 512], BF16, tag="xta")
                xt0b = xt_pool.tile([128, 8, 512], BF16, tag="xtb")
                nc.sync.dma_start(out=xt0a[:], in_=xT[:, 0, 0:8, :])
                nc.sync.dma_start(out=xt0b[:], in_=xT[:, 0, 8:16, :])

                nc.scalar.dma_start(out=cos_sb[0:64, :], in_=cosT[:])
                nc.scalar.dma_start(out=cos_sb[64:128, :], in_=cosT[:])
                nc.scalar.dma_start(out=sin_sb[64:128, :], in_=sinT[:])
                nc.vector.tensor_scalar_mul(sin_sb[0:64, :], sin_sb[64:128, :], -1.0)
                nc.scalar.dma_start(out=wv_sb[:], in_=wv[:])
                nc.scalar.dma_start(out=wq_sb[:], in_=wq[:])

                tmp = sA.enter_context(tc.tile_pool(name="tmp", bufs=3))
                nrm = sA.enter_context(tc.tile_pool(name="nrm", bufs=2))
                psk = sA.enter_context(tc.tile_pool(name="psk", bufs=1, space="PSUM"))
                psv = sA.enter_context(tc.tile_pool(name="psv", bufs=1, space="PSUM"))
                psq = sA.enter_context(tc.tile_pool(name="psq", bufs=2, space="PSUM"))
                pstr = sA.enter_context(tc.tile_pool(name="pstr", bufs=1, space="PSUM"))
                pskc = sA.enter_context(tc.tile_pool(name="pskc", bufs=1, space="PSUM"))
                psn = sA.enter_context(tc.tile_pool(name="psn", bufs=2, space="PSUM"))

                def rope_bf(dst, src_ps, tcs, tag):
                    """dst(bf16) = rope(src_ps); cast on Scalar, half-swap
                    copies + mul/mul/add on DVE in bf16."""
                    cast = tmp.tile([128, 512], BF16, tag=f"{tag}c")
                    nc.scalar.activation(out=cast[:], in_=src_ps[:], func=AF.Copy)
                    sw = tmp.tile([128, 512], BF16, tag=f"{tag}s")
                    nc.vector.tensor_copy(out=sw[0:64, :], in_=cast[64:128, :])
                    nc.vector.tensor_copy(out=sw[64:128, :], in_=cast[0:64, :])
                    t1 = tmp.tile([128, 512], BF16, tag=f"{tag}1")
                    nc.vector.tensor_mul(t1[:], cast[:], cos_sb[:, tcs])
                    nc.vector.tensor_mul(sw[:], sw[:], sin_sb[:, tcs])
                    nc.vector.tensor_add(dst, t1[:], sw[:])

                for tch in range(NTCH):
                    tcs = slice(512 * tch, 512 * tch + 512)
                    if tch == 1:
                        # non-startup-critical loads, issued once tch0 is rolling
                        nc.scalar.dma_start(
                            out=masks_sb[:], in_=masks.rearrange("d p m -> p d m")
                        )
                        nc.scalar.dma_start(out=wp_sb[:], in_=wp[:])
                    if tch == 0:
                        xta, xtb = xt0a, xt0b
                    else:
                        xta = xt_pool.tile([128, 8, 512], BF16, tag="xta")
                        xtb = xt_pool.tile([128, 8, 512], BF16, tag="xtb")
                        nc.sync.dma_start(out=xta[:], in_=xT[:, tch, 0:8, :])
                        nc.sync.dma_start(out=xtb[:], in_=xT[:, tch, 8:16, :])

                    def xt(cb):
                        return xta[:, cb, :] if cb < 8 else xtb[:, cb - 8, :]

                    # K^T chunk [128 d, 512 t]
                    ps_k = psk.tile([128, 512], F32, tag="psk")
                    for cb in range(NCB):
                        nc.tensor.matmul(
                            ps_k[:], wk_sb[:, cb, :], xt(cb),
                            start=(cb == 0), stop=(cb == NCB - 1),
                        )
                    rope_bf(kT_sb[:, tcs], ps_k, tcs, "k")
                    ksq = tmp.tile([128, 512], BF16, tag="ksq")
                    nc.vector.tensor_mul(ksq[:], kT_sb[:, tcs], kT_sb[:, tcs])

                    # V^T, then PE-transpose each 128-block into v_sb
                    ps_v = psv.tile([128, 512], F32, tag="psv")
                    for cb in range(NCB):
                        nc.tensor.matmul(
                            ps_v[:], wv_sb[:, cb, :], xt(cb),
                            start=(cb == 0), stop=(cb == NCB - 1),
                        )
                    vt_bf = tmp.tile([128, 512], BF16, tag="vtb")
                    nc.scalar.activation(out=vt_bf[:], in_=ps_v[:], func=AF.Copy)

                    # per-key rms via ksq-stationary matmuls (1 moving col)
                    for j in range(4):
                        ps_kc = pskc.tile([128, 1], F32, tag="pskc")
                        nc.tensor.matmul(
                            ps_kc[:], ksq[:, 128 * j : 128 * j + 128], ones_bf[:],
                            start=True, stop=True,
                        )
                        scol = nrm.tile([128, 1], F32, tag="scol")
                        nc.scalar.activation(
                            out=scol[:], in_=ps_kc[:], func=AF.Sqrt,
                            scale=1.0 / HD, bias=eps_k[:],
                        )
                        nc.vector.reciprocal_approx_fast(
                            out=rk_col[:, 4 * tch + j : 4 * tch + j + 1], in_=scol[:]
                        )

                    # Q^T per head [128 d, 512 t]
                    for hq in range(G):
                        ps_q = psq.tile([128, 512], F32, tag="psq")
                        for cb in range(NCB):
                            nc.tensor.matmul(
                                ps_q[:],
                                wq_sb[:, cb, 128 * hq : 128 * hq + 128],
                                xt(cb),
                                start=(cb == 0), stop=(cb == NCB - 1),
                            )
                        if hq == 0:
                            # V transposes slot in here (vt_bf is ready)
                            for tt in range(4):
                                ps_tr = pstr.tile([128, 128], BF16, tag="pstr")
                                nc.tensor.transpose(
                                    ps_tr[:], vt_bf[:, 128 * tt : 128 * tt + 128],
                                    ident_bf[:],
                                )
                                nc.vector.tensor_copy(
                                    out=v_sb[:, 4 * tch + tt, :], in_=ps_tr[:]
                                )
                        qrope = tmp.tile([128, 512], BF16, tag="qr")
                        rope_bf(qrope[:], ps_q, tcs, "q")
                        sq = tmp.tile([128, 512], BF16, tag="sq")
                        nc.vector.tensor_mul(sq[:], qrope[:], qrope[:])
                        # sum over d, broadcast to all partitions, in one matmul
                        ps_ssq = psn.tile([128, 512], F32, tag="psn")
                        nc.tensor.matmul(
                            ps_ssq[:], allones[:], sq[:], start=True, stop=True
                        )
                        # 1/(HD*rms) = 1/sqrt(HD*ss + HD^2*eps)
                        srow = nrm.tile([128, 512], F32, tag="srow")
                        nc.scalar.activation(
                            out=srow[:], in_=ps_ssq[:], func=AF.Sqrt,
                            scale=float(HD), bias=eps_q[:],
                        )
                        rcp = nrm.tile([128, 512], F32, tag="rcpq")
                        nc.vector.reciprocal_approx_fast(out=rcp[:], in_=srow[:])
                        nc.vector.tensor_mul(qT_sb[:, hq, tcs], qrope[:], rcp[:])

            # ---- phase 2: attention (head pairs), AG per chunk; phase 4 ----
            with ExitStack() as sB:
                pt_pool = sB.enter_context(tc.tile_pool(name="pt", bufs=4))
                acc_pool = sB.enter_context(tc.tile_pool(name="acc", bufs=2))
                den_pool = sB.enter_context(tc.tile_pool(name="den", bufs=2))
                yt_pool = sB.enter_context(tc.tile_pool(name="yt", bufs=1))
                ytm_pool = sB.enter_context(tc.tile_pool(name="ytm", bufs=4))
                pss = sB.enter_context(tc.tile_pool(name="pss", bufs=2, space="PSUM"))
                psy = sB.enter_context(tc.tile_pool(name="psy", bufs=4, space="PSUM"))

                yt_tiles = {}
                for c in range(3):
                    yt_tiles[(c, 0)] = yt_pool.tile(
                        [128, NCB, 512], BF16, name=f"ytall_{c}"
                    )
                for p in range(2):
                    yt_tiles[(3, p)] = yt_pool.tile(
                        [128, 8, 512], BF16, name=f"ytall_3_{p}"
                    )
                # triangle mask doubled across the head-pair dim
                mask2 = yt_pool.tile([128, 2, 128], BF16, name="mask2")
                nc.vector.tensor_copy(out=mask2[:, 0, :], in_=masks_sb[:, 0, 0:128])
                nc.vector.tensor_copy(out=mask2[:, 1, :], in_=masks_sb[:, 0, 0:128])

                chunk_order = [0, 1, 2, 3]
                for c in chunk_order:
                    nblk = 4 * c + 4
                    tqs = slice(512 * c, 512 * c + 512)
                    for pair in range(2):
                        h0, h1 = 2 * pair, 2 * pair + 1
                        ps_y0 = psy.tile([128, 512], F32, tag="psy")
                        ps_y1 = psy.tile([128, 512], F32, tag="psy")
                        accum = acc_pool.tile([128, 2, 512], BF16, tag="acc")
                        for tkb in range(nblk):
                            d = tkb - 4 * c
                            ks = slice(128 * tkb, 128 * tkb + 128)
                            # diagonal blocks d>=1: columns < 128d are fully
                            # masked; compute only the live range
                            lo = 128 * d if d >= 1 else 0
                            lv = slice(lo, 512)
                            ps_sp = pss.tile([128, 2, 512], F32, tag="pss")
                            nc.tensor.matmul(
                                ps_sp[:, 0, lv], kT_sb[:, ks],
                                qT_sb[:, h0, 512 * c + lo : 512 * c + 512],
                                start=True, stop=True,
                            )
                            nc.tensor.matmul(
                                ps_sp[:, 1, lv], kT_sb[:, ks],
                                qT_sb[:, h1, 512 * c + lo : 512 * c + 512],
                                start=True, stop=True,
                            )
                            # first exp writes the denominator accumulator tile
                            # directly; it doubles as this block's P
                            pt = (
                                accum
                                if tkb == 0
                                else pt_pool.tile([128, 2, 512], BF16, tag="pt")
                            )
                            nc.scalar.activation(
                                out=pt[:, :, lv], in_=ps_sp[:, :, lv], func=AF.Exp,
                                scale=rk_col[:, tkb : tkb + 1],
                            )
                            if d >= 0:
                                # only the leading 128 live columns need the triangle
                                ms = slice(lo, lo + 128)
                                nc.vector.tensor_mul(
                                    pt[:, :, ms], pt[:, :, ms], mask2[:]
                                )
                            if tkb > 0:
                                nc.vector.tensor_add(
                                    accum[:, :, lv], accum[:, :, lv], pt[:, :, lv]
                                )
                            nc.tensor.matmul(
                                ps_y0[:, lv], v_sb[:, tkb, :], pt[:, 0, lv],
                                start=(tkb == 0), stop=(tkb == nblk - 1),
                                skip_group_check=bool(lo),
                            )
                            nc.tensor.matmul(
                                ps_y1[:, lv], v_sb[:, tkb, :], pt[:, 1, lv],
                                start=(tkb == 0), stop=(tkb == nblk - 1),
                                skip_group_check=bool(lo),
                            )
                        # key-sum broadcast to all partitions via all-ones matmul
                        ps_d = pss.tile([128, 2, 512], F32, tag="pss")
                        nc.tensor.matmul(
                            ps_d[:, 0, :], allones[:], accum[:, 0, :],
                            start=True, stop=True,
                        )
                        nc.tensor.matmul(
                            ps_d[:, 1, :], allones[:], accum[:, 1, :],
                            start=True, stop=True,
                        )
                        rcpd = den_pool.tile([128, 2, 512], F32, tag="rcpd")
                        nc.vector.reciprocal_approx_fast(out=rcpd[:], in_=ps_d[:])
                        yT = ytm_pool.tile([128, 2, 512], BF16, tag="yT")
                        nc.vector.tensor_mul(yT[:, 0, :], ps_y0[:], rcpd[:, 0, :])
                        nc.vector.tensor_mul(yT[:, 1, :], ps_y1[:], rcpd[:, 1, :])
                        if c == 3:
                            nc.sync.dma_start(
                                out=ag_ins[(3, pair)].rearrange(
                                    "(h p) t -> p h t", p=128
                                ),
                                in_=yT[:],
                            )
                            nc.gpsimd.collective_compute(
                                "AllGather",
                                mybir.AluOpType.bypass,
                                replica_groups=[[0, 1, 2, 3], [4, 5, 6, 7]],
                                ins=[ag_ins[(3, pair)][:]],
                                outs=[ag_outs[(3, pair)][:]],
                            )
                            nc.gpsimd.dma_start(
                                out=yt_tiles[(3, pair)][:],
                                in_=ag_outs[(3, pair)].rearrange(
                                    "(i p) t -> p i t", p=128
                                ),
                            )
                        else:
                            nc.sync.dma_start(
                                out=ag_ins[(c, 0)].rearrange("(h p) t -> p h t", p=128)[
                                    :, 2 * pair : 2 * pair + 2, :
                                ],
                                in_=yT[:],
                            )
                    if c != 3:
                        nc.gpsimd.collective_compute(
                            "AllGather",
                            mybir.AluOpType.bypass,
                            replica_groups=[[0, 1, 2, 3], [4, 5, 6, 7]],
                            ins=[ag_ins[(c, 0)][:]],
                            outs=[ag_outs[(c, 0)][:]],
                        )
                        # gathered-Y load (gpsimd swdge): parks until the AG
                        # lands without blocking sync's yT writes
                        nc.gpsimd.dma_start(
                            out=yt_tiles[(c, 0)][:],
                            in_=ag_outs[(c, 0)].rearrange("(i p) t -> p i t", p=128),
                        )

                # phase 4: out^T chunk = sum_cin Wp^T y^T  (16-block PSUM chains)
                osb_pool = sB.enter_context(tc.tile_pool(name="osb", bufs=3))
                for c in chunk_order:
                    tqs = slice(512 * c, 512 * c + 512)
                    for cob in range(4):
                        ps_o = pss.tile([128, 2, 512], F32, tag="pss")
                        if c != 3:
                            srcs = [(yt_tiles[(c, 0)], i, i) for i in range(NCB)]
                        else:
                            # yt block j of pair-AG = rank j//2, head 2*pair+j%2
                            srcs = [
                                (yt_tiles[(3, p)], j, 4 * (j // 2) + 2 * p + (j % 2))
                                for p in range(2)
                                for j in range(8)
                            ]
                        for n, (yt, j, i) in enumerate(srcs):
                            nc.tensor.matmul(
                                ps_o[:, 0, :],
                                wp_sb[:, i, 128 * cob : 128 * cob + 128],
                                yt[:, j, :],
                                start=(n == 0), stop=(n == NCB - 1),
                            )
                        o_sb = osb_pool.tile([128, 512], F32, tag="osb")
                        nc.scalar.activation(out=o_sb[:], in_=ps_o[:, 0, :], func=AF.Copy)
                        nc.scalar.dma_start(
                            out=outT[128 * cob : 128 * cob + 128, tqs],
                            in_=o_sb[:],
                        )

    nc.compile()
    return nc


def _get_nc():
    if "nc" not in _CACHE:
        _CACHE["nc"] = _build()
    return _CACHE["nc"]


def _shuf_w(w):
    """[C, O] -> [128, 16*O] partition-major (contiguous per-partition runs)."""
    o = w.shape[1]
    return np.ascontiguousarray(
        w.reshape(16, 128, o).transpose(1, 0, 2).reshape(128, 16 * o)
    )


def _shuf_x(xb):
    """x[b] [T, C] -> xT [128, 4, 16, 512]: [p, tch, cb, t']."""
    xt = xb.T.reshape(16, 128, 4, 512)  # [cb, p, tch, t']
    return np.ascontiguousarray(xt.transpose(1, 2, 0, 3))


def kernel(x, cos, sin, Wq, Wk, Wv, Wp):
    from concourse.bass_utils import run_bass_kernel_spmd

    x = np.asarray(x)
    bf16 = ml_dtypes.bfloat16
    cosT = np.ascontiguousarray(np.asarray(cos).T).astype(bf16)
    sinT = np.ascontiguousarray(np.asarray(sin).T).astype(bf16)
    p = np.arange(128, dtype=np.int64)[:, None]
    j = np.arange(512, dtype=np.int64)[None, :]
    masks = np.stack(
        [(j >= p + 128 * d) for d in range(4)], axis=0
    ).astype(bf16)  # [4, 128, 512]

    in_maps = []
    for core in range(8):
        b, g = core // 4, core % 4
        in_maps.append(
            {
                "xT": _shuf_x(x[b]).astype(bf16),
                "wq": _shuf_w(Wq[:, 512 * g : 512 * g + 512]).astype(bf16),
                "wk": _shuf_w(Wk[:, 128 * g : 128 * g + 128]).astype(bf16),
                "wv": _shuf_w(Wv[:, 128 * g : 128 * g + 128]).astype(bf16),
                "wp": _shuf_w(Wp[:, 512 * g : 512 * g + 512]).astype(bf16),
                "cosT": cosT,
                "sinT": sinT,
                "masks": masks,
            }
        )

    nc = _get_nc()
    res = run_bass_kernel_spmd(nc, in_maps, core_ids=list(range(8)), trace=False)

    out = np.empty((B, T, C), dtype=np.float32)
    for core in range(8):
        b, g = core // 4, core % 4
        out[b, :, 512 * g : 512 * g + 512] = res.results[core]["outT"].T
    return out
